# revision 22
# baseline (speedup 1.0000x reference)
"""Contrastive (InfoNCE) loss kernel for Trainium2, 8 NeuronCores.

Strategy (data-parallel over z1 rows):
  - Core k owns rows [k*1024, (k+1)*1024) of view1 and receives ALL of view2,
    column-rolled by k*1024 so every core's diagonal block lands at local
    columns [0, 1024) -> a single SPMD program, no partition-id branching.
  - Host pre-transposes both operands to [D, n] layout so the contraction dim
    D sits on SBUF partitions for the PE matmuls.
  - On device, per core:
      nsq = ones[128,128].T @ x^2  (column sums, broadcast across partitions)
      inv = exp(-0.5*ln(nsq) [+ ln2])   (rsqrt via the ln/exp ACT table set;
                                         the +ln2 folds the 1/T=2 temperature)
      z = x * inv  (in-place prescale, DVE)
      sim tile [128, 1024] = z1_tile.T @ z2_tile  (bf16 matmuls, fp32 PSUM accum)
      exp+row-sum in one ACT op (no max subtraction: |logits| <= 2)
      diag extracted with an identity-mask fused multiply+row-reduce (DVE)
      row_loss = ln(sum_exp) - diag_sim
  - Host sums the 8192 per-row losses and divides by N.
"""

import numpy as np

import concourse.bass as bass
import concourse.mybir as mybir
import concourse.tile as tile
from concourse import bacc
from concourse.bass_utils import run_bass_kernel_spmd
from concourse.masks import make_identity

N, D = 8192, 1024
NC = 8
NLOC = N // NC            # rows of view1 per core
P = 128                   # SBUF partitions
KT = D // P               # contraction tiles
IT = NLOC // P            # output row tiles per core
JBW = 1024                # similarity-column block width (2 PSUM banks)
NJB = N // JBW
MMW = 512                 # fp32 moving-operand max per matmul
LN2 = 0.6931471805599453  # ln(2) == ln(1/temperature)

F32 = mybir.dt.float32
BF16 = mybir.dt.bfloat16
AF = mybir.ActivationFunctionType
ALU = mybir.AluOpType


def build_bass():
    nc = bacc.Bacc("TRN2", target_bir_lowering=False, debug=False)
    x1t = nc.dram_tensor("x1t", [D, NLOC], BF16, kind="ExternalInput")
    x2t = nc.dram_tensor("x2t", [D, N], BF16, kind="ExternalInput")
    out = nc.dram_tensor("row_loss", [P, IT], F32, kind="ExternalOutput")

    with tile.TileContext(nc) as tc:
        with (
            tc.tile_pool(name="consts", bufs=1) as consts,
            tc.tile_pool(name="x1", bufs=1) as x1pool,
            tc.tile_pool(name="x2", bufs=2) as x2pool,
            tc.tile_pool(name="sq", bufs=3) as sqpool,
            tc.tile_pool(name="nrm", bufs=2) as nrmpool,
            tc.tile_pool(name="dump", bufs=2) as dumppool,
            tc.tile_pool(name="small", bufs=1) as small,
            tc.tile_pool(name="psim", bufs=2, space="PSUM") as psim,
            tc.tile_pool(name="pnsq", bufs=2, space="PSUM") as pnsq,
        ):
            ones = consts.tile([P, P], BF16)
            nc.vector.memset(ones, 1.0)
            ident = consts.tile([P, P], F32)
            make_identity(nc, ident)
            ln2bias = consts.tile([P, 1], F32)
            nc.vector.memset(ln2bias, LN2)

            expsums = small.tile([P, IT, NJB], F32)
            diags = small.tile([P, IT], F32)

            # ---- load x1 slab, normalize columns in place (z1 = x1 * 2/||row||)
            x1s = x1pool.tile([P, KT, NLOC], BF16)
            nc.sync.dma_start(
                out=x1s, in_=x1t.ap().rearrange("(kt p) i -> p kt i", p=P)
            )
            nsq1 = pnsq.tile([P, NLOC], F32, tag="nsq")
            for kt in range(KT):
                sq = sqpool.tile([P, NLOC], BF16)
                nc.scalar.activation(sq, x1s[:, kt, :], AF.Square)
                for h in range(NLOC // MMW):
                    nc.tensor.matmul(
                        nsq1[:, h * MMW:(h + 1) * MMW],
                        ones,
                        sq[:, h * MMW:(h + 1) * MMW],
                        start=(kt == 0),
                        stop=(kt == KT - 1),
                    )
            ln1 = nrmpool.tile([P, NLOC], F32)
            nc.scalar.activation(ln1, nsq1, AF.Ln)
            n1b = nrmpool.tile([P, NLOC], BF16)
            nc.scalar.activation(n1b, ln1, AF.Exp, scale=-0.5, bias=ln2bias)
            for kt in range(KT):
                nc.vector.tensor_mul(x1s[:, kt, :], x1s[:, kt, :], n1b)

            # ---- stream x2 by column blocks
            x2r = x2t.ap().rearrange("(kt p) j -> p kt j", p=P)
            for jb in range(NJB):
                x2s = x2pool.tile([P, KT, JBW], BF16)
                nc.sync.dma_start(
                    out=x2s, in_=x2r[:, :, jb * JBW:(jb + 1) * JBW]
                )
                nsq2 = pnsq.tile([P, JBW], F32, tag="nsq")
                for kt in range(KT):
                    sq = sqpool.tile([P, JBW], BF16)
                    nc.scalar.activation(sq, x2s[:, kt, :], AF.Square)
                    for h in range(JBW // MMW):
                        nc.tensor.matmul(
                            nsq2[:, h * MMW:(h + 1) * MMW],
                            ones,
                            sq[:, h * MMW:(h + 1) * MMW],
                            start=(kt == 0),
                            stop=(kt == KT - 1),
                        )
                ln2t = nrmpool.tile([P, JBW], F32)
                nc.scalar.activation(ln2t, nsq2, AF.Ln)
                n2b = nrmpool.tile([P, JBW], BF16)
                nc.scalar.activation(n2b, ln2t, AF.Exp, scale=-0.5)
                for kt in range(KT):
                    nc.vector.tensor_mul(x2s[:, kt, :], x2s[:, kt, :], n2b)

                # ---- similarity block + online exp-sum
                for it in range(IT):
                    sim = psim.tile([P, JBW], F32)
                    for kt in range(KT):
                        for h in range(JBW // MMW):
                            nc.tensor.matmul(
                                sim[:, h * MMW:(h + 1) * MMW],
                                x1s[:, kt, it * P:(it + 1) * P],
                                x2s[:, kt, h * MMW:(h + 1) * MMW],
                                start=(kt == 0),
                                stop=(kt == KT - 1),
                            )
                    if jb == 0:
                        # diagonal block of i-tile `it` sits at columns
                        # [it*128, (it+1)*128) of the rolled layout
                        dsc = sqpool.tile([P, P], F32, tag="dsc")
                        nc.vector.tensor_mul(
                            dsc, sim[:, it * P:(it + 1) * P], ident
                        )
                        nc.vector.reduce_sum(
                            diags[:, it:it + 1], dsc,
                            axis=mybir.AxisListType.X,
                        )
                    dump = dumppool.tile([P, JBW], BF16)
                    nc.scalar.activation(
                        dump, sim, AF.Exp,
                        accum_out=expsums[:, it, jb:jb + 1],
                    )

            # ---- epilogue: row_loss = ln(sum_j exp) - diag
            s = small.tile([P, IT], F32)
            nc.vector.reduce_sum(s, expsums, axis=mybir.AxisListType.X)
            lse = small.tile([P, IT], F32)
            nc.scalar.activation(lse, s, AF.Ln)
            rl = small.tile([P, IT], F32)
            nc.vector.tensor_sub(rl, lse, diags)
            nc.sync.dma_start(out=out.ap(), in_=rl)

    nc.compile()
    return nc


_NC_CACHE = None
_LAST_RESULTS = None


def kernel(view1: np.ndarray, view2: np.ndarray) -> np.ndarray:
    global _NC_CACHE
    import ml_dtypes
    bf16 = np.dtype(ml_dtypes.bfloat16)
    x1 = np.asarray(view1, dtype=np.float32).astype(bf16)
    x2 = np.asarray(view2, dtype=np.float32).astype(bf16)
    assert x1.shape == (N, D) and x2.shape == (N, D)

    x1T = np.ascontiguousarray(x1.T)  # [D, N]
    x2T = np.ascontiguousarray(x2.T)

    in_maps = []
    for k in range(NC):
        x1t_k = np.ascontiguousarray(x1T[:, k * NLOC:(k + 1) * NLOC])
        x2t_k = np.concatenate(
            [x2T[:, k * NLOC:], x2T[:, :k * NLOC]], axis=1
        )
        in_maps.append({"x1t": x1t_k, "x2t": np.ascontiguousarray(x2t_k)})

    if _NC_CACHE is None:
        _NC_CACHE = build_bass()
    res = run_bass_kernel_spmd(_NC_CACHE, in_maps, core_ids=list(range(NC)))
    global _LAST_RESULTS
    _LAST_RESULTS = res

    total = 0.0
    for k in range(NC):
        total += res.results[k]["row_loss"].astype(np.float64).sum()
    return np.float32(total / N)
